# revision 1
# baseline (speedup 1.0000x reference)
"""ListMLE loss kernel for Trainium2 (8 NeuronCores, Bass/Tile).

loss = mean(logcumsumexp(outputs[t, labels[t]], axis=1) - outputs)

The per-row gather is done with per-partition local_scatter (GPSIMD
streams at ~line rate, unlike per-index ap_gather):
  host: counting-sort each row's labels within two position groups of
        2046 (ints only) -> run-start offsets OFF_g + sort perm PERM_g;
  device, per 128-row tile: E = exp(o) in bf16; per group g:
        T_g = local_scatter(E by OFF_g)       (E value at each run start)
        S_g = ttscan(mask*state + T_g)        (fill values through runs)
        G band = local_scatter(S_g by PERM_g) (back to original order;
                                               PERM is duplicate-free)
  the 4 tail positions (4092..4095) are gathered directly from o by
  their label values via indirect_copy (decoder-resident, no Q7 library
  swap) + mask-reduce + exp into G's tail band;
  then C = cumsum(G), ln(C) accumulated, minus sum(outputs); the 8
  per-core [128,1] partials are summed on host (the all-reduce mean).

The tile loop is software-pipelined: dma+exp for tile t+1 issue before
tile t's scatter chain so ACT's Ln(t) never blocks exp(t+1).
"""

import numpy as np

import concourse.bacc as bacc
import concourse.mybir as mybir
import concourse.tile as tile
import concourse.bass_isa as bass_isa
from concourse.bass_utils import run_bass_kernel_spmd

B, N = 8192, 4096
N_CORES = 8
ROWS = B // N_CORES          # 1024
TILES = ROWS // 128          # 8
GS = 2046                    # slots per sorted position-group
GROUPS = [(0, GS), (GS, GS)]
TAIL0 = 2 * GS               # 4092
NTAIL = N - TAIL0            # 4

_NC = None


def _local_scatter(gp, out_ap, data_ap, idxs_ap, num_elems, num_idxs):
    # like nc.gpsimd.local_scatter but allows any num_elems*32 <= 65472
    assert num_elems * 32 <= 65472 and num_elems % 2 == 0 and num_idxs % 2 == 0
    return gp.add_instruction(bass_isa.InstLocalScatter(
        name=f"I-{gp.bass.next_id()}",
        ins=[gp.lower_ap(data_ap, for_isa=True),
             gp.lower_ap(idxs_ap, for_isa=True)],
        outs=[gp.lower_ap(out_ap, for_isa=True)],
        _channels=128, _num_elems=num_elems, _num_idxs=num_idxs))


def _build(reps=1):
    nc = bacc.Bacc("TRN2", target_bir_lowering=False, debug=False,
                   num_devices=N_CORES)
    f32 = mybir.dt.float32
    bf16 = mybir.dt.bfloat16
    i16 = mybir.dt.int16
    u16 = mybir.dt.uint16
    add = mybir.AluOpType.add

    O = nc.dram_tensor("outputs", [ROWS, N], f32, kind="ExternalInput").ap()
    OFFS = [nc.dram_tensor(f"off{g}", [ROWS, N], i16,
                           kind="ExternalInput").ap() for g in range(2)]
    PERMS = [nc.dram_tensor(f"perm{g}", [ROWS, GS], i16,
                            kind="ExternalInput").ap() for g in range(2)]
    TAILW = nc.dram_tensor("tailw", [ROWS, NTAIL], u16,
                           kind="ExternalInput").ap()
    SELM = nc.dram_tensor("selmask", [128, 16 * NTAIL], f32,
                          kind="ExternalInput").ap()
    OUT = nc.dram_tensor("out", [128, 1], f32, kind="ExternalOutput").ap()

    with tile.TileContext(nc) as tc:
        with tc.tile_pool(name="dma", bufs=2) as dpool, \
             tc.tile_pool(name="cmp", bufs=2) as cpool, \
             tc.tile_pool(name="sm", bufs=1) as spool:
            # per-tile partial sums land in their own strip columns (no
            # shared-accumulator dependency chain across tiles)
            nreps = reps * TILES
            lnstrip = spool.tile([128, nreps], f32, name="lnstrip")
            ostrip = spool.tile([128, nreps], f32, name="ostrip")
            selm = spool.tile([128, 16 * NTAIL], f32, name="selm")
            nc.sync.dma_start(out=selm[:], in_=SELM[:])

            state = {}

            def front(i):
                # dma + exp + osum for pass-tile i (runs ahead of the
                # previous tile's scatter chain)
                t = i % TILES
                r0 = 128 * t
                o = dpool.tile([128, N], f32, name="o", tag="o")
                nc.sync.dma_start(out=o[:], in_=O[r0:r0 + 128, :])
                offs = [dpool.tile([128, N], i16, name=f"offt{g}",
                                   tag=f"offt{g}") for g in range(2)]
                perms = [dpool.tile([128, GS], i16, name=f"permt{g}",
                                    tag=f"permt{g}") for g in range(2)]
                for g in range(2):
                    nc.sync.dma_start(out=offs[g][:],
                                      in_=OFFS[g][r0:r0 + 128, :])
                    nc.sync.dma_start(out=perms[g][:],
                                      in_=PERMS[g][r0:r0 + 128, :])
                tw = dpool.tile([128, NTAIL], u16, name="tw", tag="tw")
                nc.sync.dma_start(out=tw[:], in_=TAILW[r0:r0 + 128, :])
                e = cpool.tile([128, N], bf16, name="e", tag="e")
                nc.scalar.activation(e[:], o[:],
                                     mybir.ActivationFunctionType.Exp)
                # sum(outputs) via ACT copy-accumulate (keeps DVE free);
                # the copy destination is scratch, reused as lnt later.
                trash = cpool.tile([128, N], bf16, name="trash", tag="lnt")
                nc.scalar.activation(trash[:], o[:],
                                     mybir.ActivationFunctionType.Copy,
                                     accum_out=ostrip[:, i:i + 1])
                state[i] = (o, e, offs, perms, tw)

            def back_pool(i):
                # tail gather + stage-1 scatters for pass-tile i; issued a
                # stage early so POOL never waits on tile i-1's fill scan
                o, e, offs, perms, tw = state.pop(i)
                G = cpool.tile([128, N], bf16, name="G", tag="G")
                # tail: gather o at the 4 tail labels (shared per-core
                # index list; select own-row entries, exp into G tail)
                traw = cpool.tile([128, 16 * NTAIL], f32, name="traw",
                                  tag="traw")
                nc.gpsimd.indirect_copy(traw[:], o[:], tw[:],
                                        i_know_ap_gather_is_preferred=True)
                tsel = cpool.tile([128, 16 * NTAIL], f32, name="tsel",
                                  tag="tsel")
                nc.vector.tensor_tensor(out=tsel[:], in0=traw[:],
                                        in1=selm[:],
                                        op=mybir.AluOpType.mult)
                tred = cpool.tile([128, NTAIL], f32, name="tred", tag="tred")
                for c in range(NTAIL):
                    nc.vector.tensor_reduce(
                        tred[:, c:c + 1], tsel[:, 16 * c:16 * (c + 1)],
                        axis=mybir.AxisListType.X, op=add)
                nc.scalar.activation(G[:, TAIL0:N], tred[:],
                                     mybir.ActivationFunctionType.Exp)

                # both groups' run-start values land in column bands of one
                # T tile; a single fused mask+fill works because each
                # group's slot 0 is always occupied (mask=0 resets the
                # recurrence at the band boundary)
                T = cpool.tile([128, TAIL0], bf16, name="T", tag="T")
                for g, (st, sz) in enumerate(GROUPS):
                    _local_scatter(nc.gpsimd, T[:, st:st + sz], e[:],
                                   offs[g][:], num_elems=sz, num_idxs=N)
                state[("b", i)] = (G, T, perms)

            def back_rest(i):
                # fill scan + stage-2 scatters + cumsum + ln for tile i
                G, T, perms = state.pop(("b", i))
                a = cpool.tile([128, TAIL0], bf16, name="a", tag="a")
                nc.vector.tensor_scalar(out=a[:], in0=T[:],
                                        scalar1=0.0, scalar2=None,
                                        op0=mybir.AluOpType.is_equal)
                S = cpool.tile([128, TAIL0], bf16, name="S", tag="S")
                nc.vector.tensor_tensor_scan(S[:], a[:], T[:], 0.0,
                                             mybir.AluOpType.mult, add)
                for g, (st, sz) in enumerate(GROUPS):
                    _local_scatter(nc.gpsimd, G[:, st:st + sz],
                                   S[:, st:st + sz], perms[g][:],
                                   num_elems=sz, num_idxs=sz)
                C = cpool.tile([128, N], bf16, name="C", tag="C")
                nc.vector.tensor_tensor_scan(C[:], G[:], G[:], 0.0, add,
                                             mybir.AluOpType.bypass)
                lnt = cpool.tile([128, N], bf16, name="lnt2", tag="lnt")
                nc.scalar.activation(lnt[:], C[:],
                                     mybir.ActivationFunctionType.Ln,
                                     accum_out=lnstrip[:, i:i + 1])

            total = reps * TILES
            front(0)
            back_pool(0)
            for i in range(total):
                if i + 1 < total:
                    front(i + 1)
                    back_pool(i + 1)
                back_rest(i)

            lnred = spool.tile([128, 1], f32, name="lnred")
            nc.vector.tensor_reduce(lnred[:], lnstrip[:],
                                    axis=mybir.AxisListType.X, op=add)
            ored = spool.tile([128, 1], f32, name="ored")
            nc.vector.tensor_reduce(ored[:], ostrip[:],
                                    axis=mybir.AxisListType.X, op=add)
            comb = spool.tile([128, 1], f32, name="comb")
            nc.vector.tensor_tensor(out=comb[:], in0=lnred[:], in1=ored[:],
                                    op=mybir.AluOpType.subtract)
            nc.sync.dma_start(out=OUT[:], in_=comb[:])
    nc.compile()
    return nc


def _get_nc():
    global _NC
    if _NC is None:
        _NC = _build()
    return _NC


def _prep_inputs(outputs, labels):
    outputs = np.ascontiguousarray(np.asarray(outputs), dtype=np.float32)
    lab = np.asarray(labels).astype(np.int16)          # values in [0, 4096)
    # one radix argsort over the first 4092 cols; key = label | group<<12
    key = lab[:, :TAIL0].copy()
    key[:, GS:] += np.int16(1 << 12)
    si_full = np.argsort(key, axis=1, kind="stable")
    sk_full = np.sort(key, axis=1, kind="stable")

    offs, perms = [], []
    for g, (st, sz) in enumerate(GROUPS):
        si = (si_full[:, st:st + sz] - st).astype(np.int16)
        SL = (sk_full[:, st:st + sz] - np.int16(g << 12)).astype(np.int16)
        off = np.full((B, N), -1, dtype=np.int16)
        # write slots in descending order so the run START wins
        slots = np.broadcast_to(
            np.arange(sz - 1, -1, -1, dtype=np.int16), (B, sz))
        np.put_along_axis(off, SL[:, ::-1].astype(np.int64), slots, axis=1)
        offs.append(off)
        perms.append(si)

    tailw = lab[:, TAIL0:].astype(np.uint16)
    selmask = np.zeros((128, 16 * NTAIL), dtype=np.float32)
    p = np.arange(128)
    for c in range(NTAIL):
        selmask[p, 16 * c + (p % 16)] = 1.0

    in_maps = []
    for c in range(N_CORES):
        sl = slice(c * ROWS, (c + 1) * ROWS)
        m = {"outputs": outputs[sl], "tailw": tailw[sl], "selmask": selmask}
        for g in range(2):
            m[f"off{g}"] = offs[g][sl]
            m[f"perm{g}"] = perms[g][sl]
        in_maps.append(m)
    return in_maps


def kernel(outputs, labels):
    nc = _get_nc()
    in_maps = _prep_inputs(outputs, labels)
    res = run_bass_kernel_spmd(nc, in_maps, core_ids=list(range(N_CORES)))
    total = sum(float(r["out"].sum()) for r in res.results)
    return np.float32(total / (B * N))

